# revision 3
# baseline (speedup 1.0000x reference)
"""Gcs pairwise-distance loss kernel v15 for Trainium2 (Bass/Tile), 8-core SPMD.

Math: with d = pred - truth viewed as [128, 512] (partition p = 4b + c,
c in {0,1} = X column chunks, {2,3} = Y) and rs[p] = rc[p] + rc[p^1] the
full group-row sum:

    sumsq[m,j] = sum_{p%4==m} (1024 d^2 - 2 rs d)  +  S2_g
    loss       = sum_{m,j} sqrt(sumsq[m,j]/4096)        (folds the /64)

All squares run on the DVE (scalar_tensor_tensor; ScalarE's Square lives in
a different ACT table set and a set switch costs ~2.7us).  ScalarE runs only
sqrt-set functions: a dependency-free dummy Sqrt at the top pulls the single
ACT table load + drain into the DMA shadow, then a Copy rescales the bias
and the real Sqrt + accumulator finishes the reduction.  Inputs are bf16
(pred on the sync HWDGE queue, truth on the scalar queue in parallel), so
the DMA moves 256KB total; d stays bf16 (tolerance is 2e-2).  S2_g comes
from the dsq accumulator via a tiny group matmul; rs from the sub's free
accumulator + stream_shuffle pair sum fused with the *(-2) into one stt.
Masks are built on-chip (iota/bitwise) under the DMA shadow; static
SBUF/PSUM allocations avoid tile-pool exit barriers.

Every core computes the full replicated result; core 0's scalar is returned.
"""

import numpy as np

_CACHE = {}


def _build_nc():
    import concourse.tile as tile
    from concourse import bacc, mybir

    f32 = mybir.dt.float32
    bf16 = mybir.dt.bfloat16
    i32 = mybir.dt.int32
    Alu = mybir.AluOpType
    Act = mybir.ActivationFunctionType
    nc = bacc.Bacc("TRN2", target_bir_lowering=False, debug=False)
    in0 = nc.dram_tensor("in0", [128, 512], bf16, kind="ExternalInput").ap()
    in1 = nc.dram_tensor("in1", [128, 512], bf16, kind="ExternalInput").ap()
    out = nc.dram_tensor("out", [1, 1], f32, kind="ExternalOutput").ap()

    tin0 = nc.alloc_sbuf_tensor("tin0", [128, 512], bf16).ap()
    tin1 = nc.alloc_sbuf_tensor("tin1", [128, 512], bf16).ap()
    td = nc.alloc_sbuf_tensor("td", [128, 512], bf16).ap()
    dsq = nc.alloc_sbuf_tensor("dsq", [128, 512], bf16).ap()
    sqp = nc.alloc_sbuf_tensor("sqp", [128, 512], bf16).ap()
    rc = nc.alloc_sbuf_tensor("rc", [128, 1], f32).ap()
    rcs = nc.alloc_sbuf_tensor("rcs", [128, 1], f32).ap()
    m2rs = nc.alloc_sbuf_tensor("m2rs", [128, 1], f32).ap()
    sqacc = nc.alloc_sbuf_tensor("sqacc", [128, 1], f32).ap()
    cm2 = nc.alloc_sbuf_tensor("cm2", [128, 1], f32).ap()
    ti1 = nc.alloc_sbuf_tensor("ti1", [128, 4], i32).ap()
    ti1b = nc.alloc_sbuf_tensor("ti1b", [128, 4], i32).ap()
    mask01 = nc.alloc_sbuf_tensor("mask01", [128, 4], bf16).ap()
    ti2 = nc.alloc_sbuf_tensor("ti2", [128, 4], i32).ap()
    ti2b = nc.alloc_sbuf_tensor("ti2b", [128, 4], i32).ap()
    maskS = nc.alloc_sbuf_tensor("maskS", [128, 4], f32).ap()
    ones4 = nc.alloc_sbuf_tensor("ones4", [4, 1], f32).ap()
    warm = nc.alloc_sbuf_tensor("warm", [4, 1], f32).ap()
    biasK_sb = nc.alloc_sbuf_tensor("biasK_sb", [4, 1], f32).ap()
    dist = nc.alloc_sbuf_tensor("dist", [4, 512], f32).ap()
    dsums = nc.alloc_sbuf_tensor("dsums", [4, 1], f32).ap()
    out_sb = nc.alloc_sbuf_tensor("out_sb", [1, 1], f32).ap()

    zeros512 = nc.alloc_sbuf_tensor("zeros512", [128, 512], bf16).ap()
    main = nc.alloc_psum_tensor("main", [4, 512], f32).ap()
    warmps = nc.alloc_psum_tensor("warmps", [4, 512], f32).ap()
    biasK = nc.alloc_psum_tensor("biasK", [4, 1], f32).ap()
    tot = nc.alloc_psum_tensor("tot", [1, 1], f32).ap()

    with tile.TileContext(nc) as tc:
        # ---- input DMAs first: pred on sync HWDGE, truth on scalar HWDGE ----
        with tc.high_priority():
            nc.sync.dma_start(tin0, in0)
            nc.scalar.dma_start(tin1, in1)

        # ---- dependency-free dummy Sqrt pulls the ACT table load early ----
        nc.gpsimd.memset(ones4, 1.0)
        nc.gpsimd.memset(zeros512, 0.0)
        nc.scalar.activation(warm, ones4, Act.Sqrt)

        # ---- on-chip constants (under the DMA shadow) ----
        nc.gpsimd.memset(cm2, -2.0)
        nc.gpsimd.iota(ti1, pattern=[[-1, 4]], base=0, channel_multiplier=1)
        nc.vector.tensor_scalar(ti1b, ti1, 3, None, op0=Alu.bitwise_and)
        # mask01[p,m] = 1.0 (bf16) iff p % 4 == m  -- main matmul lhsT
        nc.vector.tensor_scalar(mask01, ti1b, 0, None, op0=Alu.is_equal)
        nc.gpsimd.iota(ti2, pattern=[[-2, 2], [0, 2]], base=0,
                       channel_multiplier=1)
        nc.vector.tensor_scalar(ti2b, ti2, 1, 1, op0=Alu.arith_shift_right,
                                op1=Alu.bitwise_and)
        # maskS[p,m] = 1.0 (f32) iff (p>>1)&1 == m//2  -- S2 group lhsT
        nc.vector.tensor_scalar(maskS, ti2b, 0, None, op0=Alu.is_equal)

        # ---- d = pred - truth (bf16), free accum = row chunk sums ----
        nc.vector.scalar_tensor_tensor(
            out=td, in0=tin0, scalar=1.0, in1=tin1,
            op0=Alu.mult, op1=Alu.subtract, accum_out=rc,
        )
        # ---- m2rs = -2 * (rc + rc[p^1]) in two DVE ops ----
        nc.vector.stream_shuffle(rcs, rc, mask=[i ^ 1 for i in range(32)])
        nc.vector.scalar_tensor_tensor(
            out=m2rs, in0=rc, scalar=rcs, in1=cm2,
            op0=Alu.add, op1=Alu.mult,
        )
        # ---- dsq = 1024 d^2 (accum -> 1024*S2 chunks); sqp = dsq - 2 rs d --
        nc.vector.scalar_tensor_tensor(
            out=dsq, in0=td, scalar=1024.0, in1=td,
            op0=Alu.mult, op1=Alu.mult, accum_out=sqacc,
        )
        nc.vector.scalar_tensor_tensor(
            out=sqp, in0=td, scalar=m2rs, in1=dsq,
            op0=Alu.mult, op1=Alu.add,
        )

        # ---- PE p-state warmup: the tensor engine only reaches full clock
        # after ~3us of continuous execution (cost-model pstate ramp), so
        # dummy matmuls on zeroed scratch keep it busy through the DVE chain
        for _ in range(5):
            nc.tensor.matmul(warmps, mask01, zeros512, start=True, stop=True)

        # ---- PE: group bias matmul early, then batch-sum per anchor class --
        nc.tensor.matmul(biasK, maskS, sqacc, start=True, stop=True)
        # 2^-22 = 1/(1024*4096): the /1024 of S2 and the sqrt's /4096
        nc.scalar.activation(biasK_sb, biasK, Act.Copy, bias=0.0,
                             scale=1.0 / 4194304.0)
        nc.tensor.matmul(warmps[:, 0:256], mask01, zeros512[:, 0:256],
                         start=True, stop=True)
        nc.tensor.matmul(main, mask01, sqp, start=True, stop=True)

        # ---- dist = sqrt(main/4096 + S2_g/4096); accum = row sums ----
        nc.scalar.activation(dist, main, Act.Sqrt, bias=biasK_sb,
                             scale=1.0 / 4096.0, accum_out=dsums)

        # ---- total = sum_m dsums[m] via tiny PE dot; store ----
        nc.tensor.matmul(tot, ones4, dsums, start=True, stop=True)
        nc.vector.tensor_copy(out_sb, tot)
        nc.sync.dma_start(out, out_sb)

    nc.compile()
    return nc


def _get():
    if "nc" not in _CACHE:
        _CACHE["nc"] = _build_nc()
    return _CACHE["nc"]


def _in_map(pred, truth):
    import ml_dtypes

    nc = _get()
    p = np.asarray(pred, dtype=np.float32).reshape(128, 512).astype(ml_dtypes.bfloat16)
    t = np.asarray(truth, dtype=np.float32).reshape(128, 512).astype(ml_dtypes.bfloat16)
    return nc, {"in0": np.ascontiguousarray(p), "in1": np.ascontiguousarray(t)}


def kernel(pred, truth) -> np.ndarray:
    from concourse.bass_utils import run_bass_kernel_spmd

    nc, in_map = _in_map(pred, truth)
    res = run_bass_kernel_spmd(
        nc, [dict(in_map) for _ in range(8)], core_ids=list(range(8))
    )
    return res.results[0]["out"].reshape(()).astype(np.float32)


# revision 4
# speedup vs baseline: 1.1385x; 1.1385x over previous
"""Gcs pairwise-distance loss kernel v6 for Trainium2 (Bass/Tile), 8-core SPMD.

Math: with d = pred - truth viewed as [128, 512] (partition p = 4b + c,
c in {0,1} = X column chunks, {2,3} = Y) and rs[p] = rc[p] + rc[p^1] the
full group-row sum:

    sumsq[m,j] = sum_{p%4==m} (1024 d^2 - 2 rs d)  +  S2_g
    loss       = sum_{m,j} sqrt(sumsq[m,j]/4096)        (folds the /64)

All squares run on the DVE (scalar_tensor_tensor; ScalarE's Square lives in
a different ACT table set and a set switch costs ~2.7us).  ScalarE runs only
sqrt-set functions: a dependency-free dummy Sqrt at the top pulls the single
ACT table load + drain into the DMA shadow, then a Copy rescales the bias
and the real Sqrt + accumulator finishes the reduction.  Inputs are bf16
(pred on the sync HWDGE queue, truth on the scalar queue in parallel), so
the DMA moves 256KB total; d stays bf16 (tolerance is 2e-2).  S2_g comes
from the dsq accumulator via a tiny group matmul; rs from the sub's free
accumulator + stream_shuffle pair sum fused with the *(-2) into one stt.
Masks are built on-chip (iota/bitwise) under the DMA shadow; static
SBUF/PSUM allocations avoid tile-pool exit barriers.

Every core computes the full replicated result; core 0's scalar is returned.
"""

import numpy as np

_CACHE = {}


def _build_nc():
    import concourse.tile as tile
    from concourse import bacc, mybir

    f32 = mybir.dt.float32
    bf16 = mybir.dt.bfloat16
    i32 = mybir.dt.int32
    Alu = mybir.AluOpType
    Act = mybir.ActivationFunctionType
    nc = bacc.Bacc("TRN2", target_bir_lowering=False, debug=False)
    in0 = nc.dram_tensor("in0", [128, 512], bf16, kind="ExternalInput").ap()
    in1 = nc.dram_tensor("in1", [128, 512], bf16, kind="ExternalInput").ap()
    out = nc.dram_tensor("out", [1, 1], f32, kind="ExternalOutput").ap()

    tin0 = nc.alloc_sbuf_tensor("tin0", [128, 512], bf16).ap()
    tin1 = nc.alloc_sbuf_tensor("tin1", [128, 512], bf16).ap()
    td = nc.alloc_sbuf_tensor("td", [128, 512], bf16).ap()
    dsq = nc.alloc_sbuf_tensor("dsq", [128, 512], bf16).ap()
    sqp = nc.alloc_sbuf_tensor("sqp", [128, 512], bf16).ap()
    rc = nc.alloc_sbuf_tensor("rc", [128, 1], f32).ap()
    rcs = nc.alloc_sbuf_tensor("rcs", [128, 1], f32).ap()
    m2rs = nc.alloc_sbuf_tensor("m2rs", [128, 1], f32).ap()
    sqacc = nc.alloc_sbuf_tensor("sqacc", [128, 1], f32).ap()
    cm2 = nc.alloc_sbuf_tensor("cm2", [128, 1], f32).ap()
    ti1 = nc.alloc_sbuf_tensor("ti1", [128, 4], i32).ap()
    ti1b = nc.alloc_sbuf_tensor("ti1b", [128, 4], i32).ap()
    mask01 = nc.alloc_sbuf_tensor("mask01", [128, 4], bf16).ap()
    ti2 = nc.alloc_sbuf_tensor("ti2", [128, 4], i32).ap()
    ti2b = nc.alloc_sbuf_tensor("ti2b", [128, 4], i32).ap()
    maskS = nc.alloc_sbuf_tensor("maskS", [128, 4], f32).ap()
    ones4 = nc.alloc_sbuf_tensor("ones4", [4, 1], f32).ap()
    warm = nc.alloc_sbuf_tensor("warm", [4, 1], f32).ap()
    biasK_sb = nc.alloc_sbuf_tensor("biasK_sb", [4, 1], f32).ap()
    dist = nc.alloc_sbuf_tensor("dist", [4, 512], f32).ap()
    dsums = nc.alloc_sbuf_tensor("dsums", [4, 1], f32).ap()
    out_sb = nc.alloc_sbuf_tensor("out_sb", [1, 1], f32).ap()

    main = nc.alloc_psum_tensor("main", [4, 512], f32).ap()
    biasK = nc.alloc_psum_tensor("biasK", [4, 1], f32).ap()
    tot = nc.alloc_psum_tensor("tot", [1, 1], f32).ap()

    with tile.TileContext(nc) as tc:
        # ---- input DMAs first: pred on sync HWDGE, truth on scalar HWDGE ----
        with tc.high_priority():
            nc.sync.dma_start(tin0, in0)
            nc.scalar.dma_start(tin1, in1)

        # ---- dependency-free dummy Sqrt pulls the ACT table load early ----
        nc.gpsimd.memset(ones4, 1.0)
        nc.scalar.activation(warm, ones4, Act.Sqrt)

        # ---- on-chip constants (under the DMA shadow) ----
        nc.gpsimd.memset(cm2, -2.0)
        nc.gpsimd.iota(ti1, pattern=[[-1, 4]], base=0, channel_multiplier=1)
        nc.vector.tensor_scalar(ti1b, ti1, 3, None, op0=Alu.bitwise_and)
        # mask01[p,m] = 1.0 (bf16) iff p % 4 == m  -- main matmul lhsT
        nc.vector.tensor_scalar(mask01, ti1b, 0, None, op0=Alu.is_equal)
        nc.gpsimd.iota(ti2, pattern=[[-2, 2], [0, 2]], base=0,
                       channel_multiplier=1)
        nc.vector.tensor_scalar(ti2b, ti2, 1, 1, op0=Alu.arith_shift_right,
                                op1=Alu.bitwise_and)
        # maskS[p,m] = 1.0 (f32) iff (p>>1)&1 == m//2  -- S2 group lhsT
        nc.vector.tensor_scalar(maskS, ti2b, 0, None, op0=Alu.is_equal)

        # ---- d = pred - truth (bf16), free accum = row chunk sums ----
        nc.vector.scalar_tensor_tensor(
            out=td, in0=tin0, scalar=1.0, in1=tin1,
            op0=Alu.mult, op1=Alu.subtract, accum_out=rc,
        )
        # ---- m2rs = -2 * (rc + rc[p^1]) in two DVE ops ----
        nc.vector.stream_shuffle(rcs, rc, mask=[i ^ 1 for i in range(32)])
        nc.vector.scalar_tensor_tensor(
            out=m2rs, in0=rc, scalar=rcs, in1=cm2,
            op0=Alu.add, op1=Alu.mult,
        )
        # ---- dsq = 1024 d^2 (accum -> 1024*S2 chunks); sqp = dsq - 2 rs d --
        nc.vector.scalar_tensor_tensor(
            out=dsq, in0=td, scalar=1024.0, in1=td,
            op0=Alu.mult, op1=Alu.mult, accum_out=sqacc,
        )
        nc.vector.scalar_tensor_tensor(
            out=sqp, in0=td, scalar=m2rs, in1=dsq,
            op0=Alu.mult, op1=Alu.add,
        )

        # ---- PE: group bias matmul early, then batch-sum per anchor class --
        nc.tensor.matmul(biasK, maskS, sqacc, start=True, stop=True)
        # 2^-22 = 1/(1024*4096): the /1024 of S2 and the sqrt's /4096
        nc.scalar.activation(biasK_sb, biasK, Act.Copy, bias=0.0,
                             scale=1.0 / 4194304.0)
        nc.tensor.matmul(main, mask01, sqp, start=True, stop=True)

        # ---- dist = sqrt(main/4096 + S2_g/4096); accum = row sums ----
        nc.scalar.activation(dist, main, Act.Sqrt, bias=biasK_sb,
                             scale=1.0 / 4096.0, accum_out=dsums)

        # ---- total = sum_m dsums[m] via tiny PE dot; store ----
        nc.tensor.matmul(tot, ones4, dsums, start=True, stop=True)
        nc.vector.tensor_copy(out_sb, tot)
        nc.sync.dma_start(out, out_sb)

    nc.compile()
    return nc


def _get():
    if "nc" not in _CACHE:
        _CACHE["nc"] = _build_nc()
    return _CACHE["nc"]


def _in_map(pred, truth):
    import ml_dtypes

    nc = _get()
    p = np.asarray(pred, dtype=np.float32).reshape(128, 512).astype(ml_dtypes.bfloat16)
    t = np.asarray(truth, dtype=np.float32).reshape(128, 512).astype(ml_dtypes.bfloat16)
    return nc, {"in0": np.ascontiguousarray(p), "in1": np.ascontiguousarray(t)}


def kernel(pred, truth) -> np.ndarray:
    from concourse.bass_utils import run_bass_kernel_spmd

    nc, in_map = _in_map(pred, truth)
    res = run_bass_kernel_spmd(
        nc, [dict(in_map) for _ in range(8)], core_ids=list(range(8))
    )
    return res.results[0]["out"].reshape(()).astype(np.float32)
